# revision 4
# baseline (speedup 1.0000x reference)
"""DepthToSpace (cell=4, 4 split groups) Trainium2 Bass kernel.

Full input x: [8, 64, 256, 256] f32 -> output [8, 4, 1024, 1024] f32.
out[b, s, 4h+r, 4w+c] = x[b, 16s + 4r + c, h, w]

Sharding: data parallel over batch — core b handles x[b] (16.8 MB in/out).

Per-core plan (pure data movement, memory-bound):
  for s in 0..3 (split groups, double buffered):
    load  : DMA x[16s:16s+16] -> X [128p, 16ch, 2h2, 256w]   (partition p = h//2)
    shuffle: DVE strided copies  Y[p, h2, r, w, c] = X[p, 4r+c, h2, w]
    store : DMA Y -> y[s] rows 8p+4h2+r, cols 4w+c  (one contiguous 4 MB region)
Both DMA directions have contiguous >=1KB runs; the 4-byte-granularity
interleave happens on-chip where strided access is cheap.
"""

import sys

sys.path.insert(0, "/opt/trn_rl_repo")

import numpy as np

import concourse.bass as bass
import concourse.mybir as mybir
from concourse.bass_utils import run_bass_kernel_spmd

B, C, H, W = 8, 64, 256, 256
S = 4
CELL = 4  # sqrt(C // S)
CPG = C // S  # channels per group = CELL^2 = 16
P = 128  # SBUF partitions
HB = H // P  # h rows per partition = 2
N_CORES = 8


def build_program():
    nc = bass.Bass()
    x = nc.declare_dram_parameter("x", [C, H, W], mybir.dt.float32, isOutput=False)
    y = nc.declare_dram_parameter(
        "y", [S, H * CELL, W * CELL], mybir.dt.float32, isOutput=True
    )

    with (
        nc.sbuf_tensor([P, CPG, HB, W], mybir.dt.float32) as X0,
        nc.sbuf_tensor([P, CPG, HB, W], mybir.dt.float32) as X1,
        nc.sbuf_tensor([P, HB, CELL, W, CELL], mybir.dt.float32) as Y0,
        nc.sbuf_tensor([P, HB, CELL, W, CELL], mybir.dt.float32) as Y1,
        nc.semaphore("in_sem0") as in_sem0,
        nc.semaphore("in_sem1") as in_sem1,
        nc.semaphore("out_sem0") as out_sem0,
        nc.semaphore("out_sem1") as out_sem1,
        nc.semaphore("shuf_sem") as shuf_sem,
        nc.Block() as block,
    ):
        Xt = [X0, X1]
        Yt = [Y0, Y1]
        in_sem = [in_sem0, in_sem1]
        out_sem = [out_sem0, out_sem1]

        def load_ap(s):
            # DRAM side: per (p, ch) a 2KB contiguous run (rows 2p, 2p+1)
            return x[s * CPG : (s + 1) * CPG].rearrange(
                "c (p h2) w -> p c h2 w", h2=HB
            )

        def store_ap(s):
            # y[s] as [p, h2, r, w, c]: row = 8p + 4h2 + r, col = 4w + c
            return y[s].rearrange(
                "(p h2 r) (w c) -> p h2 r w c", h2=HB, r=CELL, c=CELL
            )

        @block.sync
        def _(sync):
            for s in range(S):
                if s >= 2:
                    # X[s%2] is free once shuffle(s-2) is fully done
                    sync.wait_ge(shuf_sem, HB * (s - 1))
                sync.dma_start(out=Xt[s % 2][:], in_=load_ap(s)).then_inc(
                    in_sem[s % 2], 16
                )

        @block.vector
        def _(vector):
            for s in range(S):
                vector.wait_ge(in_sem[s % 2], 16 * (s // 2 + 1))
                if s >= 2:
                    # Y[s%2] is free once store(s-2) is done
                    vector.wait_ge(out_sem[s % 2], 16 * (s // 2))
                xin = Xt[s % 2][:].rearrange(
                    "p (r c) h2 w -> p r c h2 w", r=CELL, c=CELL
                )
                for h2 in range(HB):
                    src = xin[:, :, :, h2, :]
                    # Y[:, h2] is [p, r, w, c]; iterate (p, r, c, w) on both sides
                    dst = Yt[s % 2][:, h2].transpose([0, 1, 3, 2])
                    vector.tensor_copy(out=dst, in_=src).then_inc(shuf_sem, 1)

        @block.scalar
        def _(scalar):
            for s in range(S):
                scalar.wait_ge(shuf_sem, HB * (s + 1))
                scalar.dma_start(out=store_ap(s), in_=Yt[s % 2][:]).then_inc(
                    out_sem[s % 2], 16
                )
            scalar.wait_ge(out_sem0, 16 * (S // 2))
            scalar.wait_ge(out_sem1, 16 * (S // 2))

    return nc


def run_sharded(x: np.ndarray, trace: bool = False):
    """Shard x over batch across 8 cores, run, gather. Returns (out, results)."""
    assert x.shape == (B, C, H, W), x.shape
    nc = build_program()
    in_maps = [{"x": np.ascontiguousarray(x[b])} for b in range(N_CORES)]
    res = run_bass_kernel_spmd(nc, in_maps, list(range(N_CORES)), trace=trace)
    out = np.stack([res.results[b]["y"] for b in range(N_CORES)], axis=0)
    return out.astype(x.dtype, copy=False), res


def kernel(**inputs: np.ndarray) -> np.ndarray:
    x = np.asarray(inputs["x"], dtype=np.float32)
    out, _ = run_sharded(x, trace=False)
    return out
